# revision 10
# baseline (speedup 1.0000x reference)
"""Trainium2 Bass kernel for nn_AttentionV2 (dense transformer attention block).

Reference computation (per batch element b):
    q  = Wq @ x_b  + qb          # [128, 4096]  (1x1 conv over channels)
    k  = Wk @ aux_b + kb         # [128, 4096]
    v  = Wv @ aux_b + vb         # [128, 4096]
    ktq[i, j] = sum_c k[c, i] * q[c, j]          # [4096, 4096]
    atten = softmax(ktq, axis=j)
    y[c, j] = sum_i v[c, i] * atten[i, j]        # [128, 4096]
    z = Wz @ y + zb + x_b        # [256, 4096]

Sharding: batch B=8 across the 8 cores (data parallel, weights replicated).
Each core runs the whole attention for its batch element; no collectives.

v2 design (constants measured on hw via a probe kernel):
  * PE streams 512-col matmuls at 2.4GHz (216ns) for 16-bit dtypes and
    fp32r with LDWEIGHTS fully hidden; raw fp32 runs 2-pass (429ns).  The
    two big matmuls (ktq, y) are 256 x 512-col each -> ~112us PE floor;
    ScalarE exp of all 16.8M logits is ~150-160us -> ACT is the bottleneck
    and the PE must never let it starve.
  * q/k/exp/vts/y all fp16 (flushes cast); convs run on raw fp32 inputs
    (2-pass PE) so no input casts exist at all (GpSimd casts measured
    3.5ns/elem - useless; DVE cycles are precious).
  * softmax is unnormalized (shift -17 keeps exp in fp16 range; max logit
    ~26) with the row-sum reciprocal folded into vts; row sums come from
    ACT accum_out (+182ns/chunk measured; a DVE fp16 reduce gets no 2x
    mode and would cost 4.4us/tile).
  * psum: 2 x [128,1536] rotating ktq chunk slots (6 banks) feed exp;
    2 x [128,512] slots (2 banks) shared by y accumulation, convs, the
    bias-broadcast builders and the z tail.
  * i-tiles in 8 groups of 4 (one 512-col aux chunk per group); y matmuls
    for group g run during g+1, interleaved between ktq chunks so the PE
    queue never head-of-line blocks on a psum-slot wait.
  * all biases are applied on DVE during psum->sbuf flushes; per-partition
    bias columns are built as [128,w] broadcast tiles by K=1 matmuls from
    contiguous [1,128] bias rows (no scatter DMAs, no ACT identity work).
  * x conv chunks stay resident in SBUF and are reused for the +x residual
    (saves 4MB of tail DMA).
"""

import sys

if "/opt/trn_rl_repo" not in sys.path:
    sys.path.insert(0, "/opt/trn_rl_repo")

import numpy as np

import concourse.bass as bass
import concourse.bacc as bacc
import concourse.mybir as mybir
import concourse.tile as tile

DT = mybir.dt.float32
F16 = mybir.dt.float16
P = 128          # partitions
C = 256          # input channels
CH = 128         # conv output channels (C//2)
HW = 4096        # 64*64 spatial
NJB = HW // 512  # 8 column blocks of 512
NIT = HW // P    # 32 i-tiles
G = 4            # i-tiles per group == i-tiles per 512-col aux chunk
NG = NIT // G    # 8 groups
EXP_CHUNKS = ((0, 1536), (1536, 1536), (3072, 1024))
EXP_BUFS = 10
VTS_BUFS = 10
EXP_SHIFT = -17.0

Exp = mybir.ActivationFunctionType.Exp
AX = mybir.AxisListType.X


def build_module(debug: bool = False) -> bass.Bass:
    nc = bacc.Bacc("TRN2", target_bir_lowering=False)

    x = nc.declare_dram_parameter("x", [C, HW], DT, isOutput=False)
    aux = nc.declare_dram_parameter("aux", [C, HW], DT, isOutput=False)
    # conv weights arrive pre-transposed from the host (numpy .T is free)
    WqT_d = nc.declare_dram_parameter("WqT_d", [C, CH], DT, isOutput=False)
    Wq_b = nc.declare_dram_parameter("Wq_b", [CH], DT, isOutput=False)
    WkT_d = nc.declare_dram_parameter("WkT_d", [C, CH], DT, isOutput=False)
    Wk_b = nc.declare_dram_parameter("Wk_b", [CH], DT, isOutput=False)
    WvT_d = nc.declare_dram_parameter("WvT_d", [C, CH], DT, isOutput=False)
    Wv_b = nc.declare_dram_parameter("Wv_b", [CH], DT, isOutput=False)
    WzT_d = nc.declare_dram_parameter("WzT_d", [CH, C], DT, isOutput=False)
    Wz_b = nc.declare_dram_parameter("Wz_b", [C], DT, isOutput=False)
    z = nc.declare_dram_parameter("z", [C, HW], DT, isOutput=True)
    dbg = {}
    if debug:
        for nm, shape in (("dq", [P, HW]), ("dk", [P, HW]), ("dvT", [P, HW]),
                          ("dy", [P, HW]), ("dexp0", [P, HW]), ("dsums", [P, NIT])):
            dbg[nm] = nc.declare_dram_parameter(nm, shape, DT, isOutput=True)

    with tile.TileContext(nc) as tc:
        with (
            tc.tile_pool(name="consts", bufs=1) as consts,
            tc.tile_pool(name="sing", bufs=1) as sing,
            tc.tile_pool(name="expp", bufs=EXP_BUFS) as expp,
            tc.tile_pool(name="vtsp", bufs=VTS_BUFS) as vtsp,
            tc.tile_pool(name="instream", bufs=6) as instream,
            tc.tile_pool(name="wload", bufs=1) as wload,
            tc.tile_pool(name="smalls", bufs=8) as smalls,
            tc.tile_pool(name="zst", bufs=3) as zst,
            tc.tile_pool(name="psK", bufs=2, space="PSUM") as psK,
            tc.tile_pool(name="psY", bufs=2, space="PSUM") as psY,
        ):
            # ---- head DMA stream (sync queue is FIFO: order = priority;
            #      the chain to the first exp is Wq,x0,qb -> Wk,aux0,kb) ----
            wts: dict[str, bass.AP] = {}

            def emit_w_dma(name, w_dram):
                wt = wload.tile([P, 2, P], DT, tag="wl" + name)
                for h in range(2):
                    nc.sync.dma_start(out=wt[:, h], in_=w_dram[h * P : (h + 1) * P, :])
                wts[name] = wt

            # x chunks stay resident: conv input now, +x residual at the tail
            xch: list = [None] * NJB

            xh_t: dict[int, tuple] = {}

            def emit_x_dma(cb: int, eng=None) -> None:
                js = cb * 512
                eng = eng or nc.sync
                x0 = sing.tile([P, 512], DT, name=f"x0_{cb}")
                eng.dma_start(out=x0, in_=x[0:P, js : js + 512])
                x1 = sing.tile([P, 512], DT, name=f"x1_{cb}")
                eng.dma_start(out=x1, in_=x[P:C, js : js + 512])
                xch[cb] = (x0, x1)

            def emit_x_cast(cb: int) -> None:
                x0, x1 = xch[cb]
                h0 = instream.tile([P, 512], F16, tag="xh", bufs=6)
                nc.vector.tensor_copy(h0, x0)
                h1 = instream.tile([P, 512], F16, tag="xh", bufs=6)
                nc.vector.tensor_copy(h1, x1)
                xh_t[cb] = (h0, h1)

            aux_t: dict[int, tuple] = {}

            ah_t: dict[int, tuple] = {}

            def emit_aux_dma(ac: int) -> None:
                a0 = instream.tile([P, 512], DT, tag="ains", bufs=6)
                nc.sync.dma_start(out=a0, in_=aux[0:P, ac * 512 : ac * 512 + 512])
                a1 = instream.tile([P, 512], DT, tag="ains", bufs=6)
                nc.sync.dma_start(out=a1, in_=aux[P:C, ac * 512 : ac * 512 + 512])
                aux_t[ac] = (a0, a1)

            def emit_aux_cast(ac: int) -> None:
                a0, a1 = aux_t[ac]
                h0 = instream.tile([P, 512], F16, tag="ah", bufs=6)
                nc.vector.tensor_copy(h0, a0)
                h1 = instream.tile([P, 512], F16, tag="ah", bufs=6)
                nc.vector.tensor_copy(h1, a1)
                ah_t[ac] = (h0, h1)

            emit_w_dma("q", WqT_d)
            emit_x_dma(0, eng=nc.scalar)
            emit_x_dma(1, eng=nc.scalar)
            emit_x_dma(2, eng=nc.scalar)
            qb_row = consts.tile([1, P], DT)
            nc.sync.dma_start(out=qb_row, in_=Wq_b[:].rearrange("(o p) -> o p", o=1))
            emit_w_dma("k", WkT_d)
            emit_aux_dma(0)
            kb_row = consts.tile([1, P], DT)
            nc.sync.dma_start(out=kb_row, in_=Wk_b[:].rearrange("(o p) -> o p", o=1))
            vb_row = consts.tile([1, P], DT)
            nc.sync.dma_start(out=vb_row, in_=Wv_b[:].rearrange("(o p) -> o p", o=1))
            emit_w_dma("v", WvT_d)
            wtz = wload.tile([P, C], DT, tag="wlz")
            nc.sync.dma_start(out=wtz, in_=WzT_d[:, :])
            for cb in range(3, NJB):
                emit_x_dma(cb)
            zb_row = consts.tile([1, C], DT)
            nc.sync.dma_start(out=zb_row, in_=Wz_b[:].rearrange("(o p) -> o p", o=1))
            emit_aux_dma(1)
            emit_aux_dma(2)

            # ---- consts (fp16 rows so the K=1 broadcast matmuls run
            #      1-pass; fp32 K=1 LOW_HIGH measured 13.5us for 5 tiles) ----
            ones512h = consts.tile([1, 512], F16)
            nc.vector.memset(ones512h, 1.0)
            eshift = consts.tile([P, 1], DT)
            nc.vector.memset(eshift, EXP_SHIFT)

            wts16: dict[str, bass.AP] = {}
            w16 = consts.tile([P, 2, P], F16, name="w16q")
            nc.vector.tensor_copy(w16, wts["q"])
            wts16["q"] = w16

            def row16(row_ap, width: int, name: str):
                r = consts.tile([1, width], F16, name=name)
                nc.vector.tensor_copy(r, row_ap)
                return r

            # bias broadcast tiles via K=1 matmuls (fp16 inputs, fp32 out).
            # per-partition ([128,1]-style) biases: bc[p, j] = bias[p]
            #   -> stationary = bias row, moving = ones row.
            # per-column (vT's c bias): bc[p, j] = bias[j]
            #   -> stationary = ones row, moving = bias row.
            def emit_bcast(stat_row, mov_row, width: int, name: str):
                ps = psY.tile([P, width], DT, tag="y")
                nc.tensor.matmul(ps, stat_row, mov_row[:, 0:width],
                                 start=True, stop=True)
                t = consts.tile([P, width], DT, name=name)
                nc.vector.tensor_copy(t, ps)
                return t

            qb_bc = emit_bcast(row16(qb_row, P, "qb16"), ones512h, 512, "qb_bc")
            for wname in ("k", "v"):
                w16 = consts.tile([P, 2, P], F16, name="w16" + wname)
                nc.vector.tensor_copy(w16, wts[wname])
                wts16[wname] = w16
            kb_bc = emit_bcast(row16(kb_row, P, "kb16"), ones512h, 512, "kb_bc")
            ones_row_h = consts.tile([1, P], F16)
            nc.vector.memset(ones_row_h, 1.0)
            vb_bc = emit_bcast(ones_row_h, row16(vb_row, P, "vb16"), P, "vb_bc")

            # late consts, emitted as fills inside group 1 (off the
            # critical path of the first exp)
            late: dict[str, bass.AP] = {}

            def emit_late_consts() -> None:
                late["WzT"] = consts.tile([P, 2, P], F16, name="WzT")
                nc.vector.tensor_copy(late["WzT"], wtz.rearrange("p (t q) -> p t q", t=2))
                late["zb_bc0"] = emit_bcast(
                    row16(zb_row[:, 0:P], P, "zb16_0"), ones512h, 512, "zb_bc0")
                late["zb_bc1"] = emit_bcast(
                    row16(zb_row[:, P:C], P, "zb16_1"), ones512h, 512, "zb_bc1")

            # x + zb precombine, in place on the idle GpSimd engine (x raw is
            # only needed by the q convs, all emitted in group 0): the z tail
            # then needs a single DVE add per psum flush.
            def emit_xzb(cb: int) -> None:
                x0, x1 = xch[cb]
                nc.gpsimd.tensor_add(x0, x0, late["zb_bc0"])
                nc.gpsimd.tensor_add(x1, x1, late["zb_bc1"])

            # ---- persistent operands ----
            q_sb = sing.tile([P, HW], F16)
            k_sb = sing.tile([P, HW], F16)
            vT_sb = sing.tile([P, HW], F16)   # 32 tiles of [i=128, c=128]
            y_sb = sing.tile([P, HW], F16)
            # softmax row sums: persistent so exp ACTIVATE carries no
            # pool-slot cross-engine dependency
            sums = sing.tile([P, NIT, len(EXP_CHUNKS)], DT)

            # ---- conv emitters (raw fp32, 2-pass PE; bias folded into the
            #      DVE flush) ----
            def emit_q(cb: int) -> None:
                js = cb * 512
                x0, x1 = xh_t[cb]
                qp = psY.tile([P, 512], DT, tag="y")
                nc.tensor.matmul(qp, wts16["q"][:, 0], x0, start=True, stop=False)
                nc.tensor.matmul(qp, wts16["q"][:, 1], x1, start=False, stop=True)
                nc.vector.tensor_add(q_sb[:, js : js + 512], qp, qb_bc)

            def emit_k(ac: int) -> None:
                js = ac * 512
                a0, a1 = ah_t[ac]
                kp = psY.tile([P, 512], DT, tag="y")
                nc.tensor.matmul(kp, wts16["k"][:, 0], a0, start=True, stop=False)
                nc.tensor.matmul(kp, wts16["k"][:, 1], a1, start=False, stop=True)
                nc.vector.tensor_add(k_sb[:, js : js + 512], kp, kb_bc)

            def emit_v(ac: int, half: int) -> None:
                # vT[i, c] for the 2 i-tiles in `half` of aux chunk ac
                a0, a1 = ah_t[ac]
                for ti in range(2):
                    t = half * 2 + ti
                    vp = psY.tile([P, P], DT, tag="y")
                    nc.tensor.matmul(vp, a0[:, t * P : (t + 1) * P], wts16["v"][:, 0],
                                     start=True, stop=False)
                    nc.tensor.matmul(vp, a1[:, t * P : (t + 1) * P], wts16["v"][:, 1],
                                     start=False, stop=True)
                    off = ac * 512 + t * P
                    nc.vector.tensor_add(vT_sb[:, off : off + P], vp, vb_bc)

            # ---- attention emitters ----
            exp_t: dict[int, bass.AP] = {}
            vts_t: dict[int, bass.AP] = {}

            def emit_ktq_chunk(it: int, ci: int) -> None:
                if ci == 0:
                    exp_t[it] = expp.tile([P, HW], F16, tag="exp", name="et")
                off, w = EXP_CHUNKS[ci]
                kt = psK.tile([P, w], DT, tag="kt")
                for s in range(w // 512):
                    nc.tensor.matmul(
                        kt[:, s * 512 : (s + 1) * 512],
                        k_sb[:, it * P : (it + 1) * P],
                        q_sb[:, off + s * 512 : off + (s + 1) * 512],
                        start=True, stop=True,
                    )
                nc.scalar.activation(
                    out=exp_t[it][:, off : off + w], in_=kt, func=Exp,
                    bias=eshift, scale=1.0,
                    accum_out=sums[:, it, ci : ci + 1],
                )

            def emit_fin(it: int) -> None:
                sv = smalls.tile([P, 1], DT, tag="sv")
                nc.vector.reduce_sum(sv, sums[:, it], axis=AX)
                rv = smalls.tile([P, 1], DT, tag="rv")
                nc.vector.reciprocal(rv, sv)
                vt = vtsp.tile([P, P], F16, tag="vts")
                nc.vector.tensor_scalar_mul(vt, vT_sb[:, it * P : (it + 1) * P], rv)
                vts_t[it] = vt

            def emit_y(g: int, jb: int) -> None:
                """y[:, jb] += vts.T @ exp over the 4 i-tiles of group g."""
                js = jb * 512
                yp = psY.tile([P, 512], DT, tag="y")
                for gi in range(G):
                    it = g * G + gi
                    nc.tensor.matmul(
                        yp, vts_t[it], exp_t[it][:, js : js + 512],
                        start=(gi == 0), stop=(gi == G - 1),
                    )
                if g == 0:
                    nc.vector.tensor_copy(y_sb[:, js : js + 512], yp)
                else:
                    nc.vector.tensor_add(
                        y_sb[:, js : js + 512], y_sb[:, js : js + 512], yp
                    )

            def emit_z(jb: int) -> None:
                # tail-only: ScalarE is idle after the last exp, so it does
                # the psum->sbuf flush; the +((x+zb)) add is split DVE/GpSimd
                js = jb * 512
                xzb = xch[jb]
                for h in range(2):
                    zp = psK.tile([P, 512], DT, tag="kt")
                    nc.tensor.matmul(zp, late["WzT"][:, h], y_sb[:, js : js + 512],
                                     start=True, stop=True)
                    zc = zst.tile([P, 512], DT, tag="zc")
                    nc.scalar.copy(zc, zp)
                    eng = nc.vector if h == 0 else nc.gpsimd
                    eng.tensor_add(zc, zc, xzb[h])
                    nc.sync.dma_start(out=z[h * P : (h + 1) * P, js : js + 512], in_=zc)

            # ================= schedule =================
            # warmup = group 0: q cols first (ktq moving operand), k/v for
            # group 0 just-in-time, interleaved with tile 0-3 ktq/exp; convs
            # for group 1 land inside group 0's stream.
            emit_x_cast(0)
            emit_q(0)
            emit_aux_cast(0)
            emit_k(0)
            emit_x_cast(1)
            emit_q(1)
            emit_x_cast(2)
            emit_q(2)
            emit_v(0, 0)
            emit_v(0, 1)
            emit_ktq_chunk(0, 0)
            emit_x_cast(3)
            emit_q(3)
            emit_x_cast(4)
            emit_q(4)
            emit_x_cast(5)
            emit_q(5)
            emit_ktq_chunk(0, 1)
            emit_x_cast(6)
            emit_q(6)
            emit_x_cast(7)
            emit_q(7)
            emit_ktq_chunk(0, 2)
            emit_fin(0)
            emit_ktq_chunk(1, 0)
            emit_aux_cast(1)
            emit_k(1)
            emit_ktq_chunk(1, 1)
            emit_v(1, 0)
            emit_ktq_chunk(1, 2)
            emit_fin(1)
            emit_v(1, 1)
            for it in (2, 3):
                for ci in range(3):
                    emit_ktq_chunk(it, ci)
                emit_fin(it)

            # steady state: group g runs its 12 ktq/exp chunks with fills:
            # y(g-1) x8, conv k/v for group g+1, aux prefetch for g+2.
            for g in range(1, NG):
                fills: list = []
                if g == 1:
                    fills.append(emit_late_consts)
                if 2 <= g <= 5:
                    for cb in (2 * g - 4, 2 * g - 3):
                        fills.append(lambda c=cb: emit_xzb(c))
                if g + 2 < NG:
                    fills.append(lambda a=g + 2: emit_aux_dma(a))
                fills.append(lambda gg=g: emit_y(gg - 1, 0))
                if g + 1 < NG:
                    fills.append(lambda a=g + 1: emit_aux_cast(a))
                    fills.append(lambda a=g + 1: emit_k(a))
                fills.append(lambda gg=g: emit_y(gg - 1, 1))
                if g + 1 < NG:
                    fills.append(lambda a=g + 1: emit_v(a, 0))
                fills.append(lambda gg=g: emit_y(gg - 1, 2))
                if g + 1 < NG:
                    fills.append(lambda a=g + 1: emit_v(a, 1))
                for jb in range(3, NJB):
                    fills.append(lambda gg=g, j=jb: emit_y(gg - 1, j))
                fi = 0
                nslots = G * 3
                nf = len(fills)
                acc = 0.0
                for t in range(G):
                    it = g * G + t
                    for ci in range(3):
                        emit_ktq_chunk(it, ci)
                        acc += nf / nslots
                        while fi < nf and fi < acc:
                            fills[fi]()
                            fi += 1
                    emit_fin(it)
                while fi < nf:
                    fills[fi]()
                    fi += 1

            # tail: y for the last group, z streamed per column block
            emit_y(NG - 1, 0)
            for jb in range(1, NJB):
                emit_y(NG - 1, jb)
                emit_z(jb - 1)
            emit_z(NJB - 1)
            if debug:
                def dump(dst, src_ap):
                    t = zst.tile([P, 512], DT, tag="zc")
                    nc.vector.tensor_copy(t, src_ap)
                    nc.sync.dma_start(out=dst, in_=t)
                for cb in range(NJB):
                    sl = slice(cb * 512, cb * 512 + 512)
                    dump(dbg["dq"][:, sl], q_sb[:, sl])
                    dump(dbg["dk"][:, sl], k_sb[:, sl])
                    dump(dbg["dvT"][:, sl], vT_sb[:, sl])
                    dump(dbg["dy"][:, sl], y_sb[:, sl])
                    dump(dbg["dexp0"][:, sl], exp_t[0][:, sl])
                sv2 = smalls.tile([P, NIT], DT, tag="dbg2")
                for it in range(NIT):
                    nc.vector.reduce_sum(sv2[:, it : it + 1], sums[:, it], axis=AX)
                nc.sync.dma_start(out=dbg["dsums"][:, :], in_=sv2)

    nc.compile()
    return nc


_NC = None


def _get_nc() -> bass.Bass:
    global _NC
    if _NC is None:
        _NC = build_module()
    return _NC


def _make_in_maps(inputs: dict[str, np.ndarray]) -> list[dict[str, np.ndarray]]:
    B = inputs["x"].shape[0]
    shared = {
        name: np.ascontiguousarray(np.asarray(inputs[name], dtype=np.float32))
        for name in ("Wq_b", "Wk_b", "Wv_b", "Wz_b")
    }
    for dev_name, host_name in (
        ("WqT_d", "Wq_w"), ("WkT_d", "Wk_w"), ("WvT_d", "Wv_w"), ("WzT_d", "Wz_w"),
    ):
        shared[dev_name] = np.ascontiguousarray(
            np.asarray(inputs[host_name], dtype=np.float32).T
        )
    in_maps = []
    for b in range(B):
        m = dict(shared)
        m["x"] = np.ascontiguousarray(
            np.asarray(inputs["x"][b], dtype=np.float32).reshape(C, HW)
        )
        m["aux"] = np.ascontiguousarray(
            np.asarray(inputs["aux"][b], dtype=np.float32).reshape(C, HW)
        )
        in_maps.append(m)
    return in_maps


def _install_ntff_hook_shim() -> None:
    """The agent image's antenv lacks axon_hooks; recreate it so
    run_bass_kernel_spmd(trace=True) can reach the libaxon NTFF profiler."""
    import types

    if "antenv.axon_hooks" in sys.modules:
        return
    import antenv

    mod = types.ModuleType("antenv.axon_hooks")
    state = {"hook": None}
    mod.set_axon_ntff_profile_hook = lambda h: state.__setitem__("hook", h)
    mod.get_axon_ntff_profile_hook = lambda: state["hook"]
    sys.modules["antenv.axon_hooks"] = mod
    antenv.axon_hooks = mod
    try:
        from trn_agent_boot.trn_boot import _ntff_profile_via_ctypes

        hook = _ntff_profile_via_ctypes("/opt/axon/libaxon_pjrt.so")
        if hook is not None:
            mod.set_axon_ntff_profile_hook(hook)
    except Exception as e:  # degrade to no tracing
        print(f"ntff hook unavailable: {e}", file=sys.stderr)


def run(inputs: dict[str, np.ndarray], trace: bool = False):
    """Run on the 8 NeuronCores; returns (output [8,256,64,64], BassKernelResults)."""
    from concourse.bass_utils import run_bass_kernel_spmd

    if trace:
        _install_ntff_hook_shim()
    nc = _get_nc()
    in_maps = _make_in_maps(inputs)
    res = run_bass_kernel_spmd(nc, in_maps, list(range(len(in_maps))), trace=trace)
    out = np.stack([r["z"].reshape(C, 64, 64) for r in res.results])
    return out.astype(np.float32), res


def kernel(**inputs: np.ndarray) -> np.ndarray:
    out, _ = run(inputs, trace=False)
    return out


if __name__ == "__main__":
    nc = build_module()
    print("module built ok")


# revision 11
# speedup vs baseline: 1.0373x; 1.0373x over previous
"""Trainium2 Bass kernel for nn_AttentionV2 (dense transformer attention block).

Reference computation (per batch element b):
    q  = Wq @ x_b  + qb          # [128, 4096]  (1x1 conv over channels)
    k  = Wk @ aux_b + kb         # [128, 4096]
    v  = Wv @ aux_b + vb         # [128, 4096]
    ktq[i, j] = sum_c k[c, i] * q[c, j]          # [4096, 4096]
    atten = softmax(ktq, axis=j)
    y[c, j] = sum_i v[c, i] * atten[i, j]        # [128, 4096]
    z = Wz @ y + zb + x_b        # [256, 4096]

Sharding: batch B=8 across the 8 cores (data parallel, weights replicated).
Each core runs the whole attention for its batch element; no collectives.

v2 design (constants measured on hw via a probe kernel):
  * PE streams 512-col matmuls at 2.4GHz (216ns) for 16-bit dtypes and
    fp32r with LDWEIGHTS fully hidden; raw fp32 runs 2-pass (429ns).  The
    two big matmuls (ktq, y) are 256 x 512-col each -> ~112us PE floor;
    ScalarE exp of all 16.8M logits is ~150-160us -> ACT is the bottleneck
    and the PE must never let it starve.
  * q/k/exp/vts/y all fp16 (flushes cast); convs run on raw fp32 inputs
    (2-pass PE) so no input casts exist at all (GpSimd casts measured
    3.5ns/elem - useless; DVE cycles are precious).
  * softmax is unnormalized (shift -17 keeps exp in fp16 range; max logit
    ~26) with the row-sum reciprocal folded into vts; row sums come from
    ACT accum_out (+182ns/chunk measured; a DVE fp16 reduce gets no 2x
    mode and would cost 4.4us/tile).
  * psum: 2 x [128,1536] rotating ktq chunk slots (6 banks) feed exp;
    2 x [128,512] slots (2 banks) shared by y accumulation, convs, the
    bias-broadcast builders and the z tail.
  * i-tiles in 8 groups of 4 (one 512-col aux chunk per group); y matmuls
    for group g run during g+1, interleaved between ktq chunks so the PE
    queue never head-of-line blocks on a psum-slot wait.
  * all biases are applied on DVE during psum->sbuf flushes; per-partition
    bias columns are built as [128,w] broadcast tiles by K=1 matmuls from
    contiguous [1,128] bias rows (no scatter DMAs, no ACT identity work).
  * x conv chunks stay resident in SBUF and are reused for the +x residual
    (saves 4MB of tail DMA).
"""

import sys

if "/opt/trn_rl_repo" not in sys.path:
    sys.path.insert(0, "/opt/trn_rl_repo")

import numpy as np

import concourse.bass as bass
import concourse.bacc as bacc
import concourse.mybir as mybir
import concourse.tile as tile

DT = mybir.dt.float32
F16 = mybir.dt.float16
P = 128          # partitions
C = 256          # input channels
CH = 128         # conv output channels (C//2)
HW = 4096        # 64*64 spatial
NJB = HW // 512  # 8 column blocks of 512
NIT = HW // P    # 32 i-tiles
G = 4            # i-tiles per group == i-tiles per 512-col aux chunk
NG = NIT // G    # 8 groups
EXP_CHUNKS = ((0, 1536), (1536, 1536), (3072, 1024))
EXP_BUFS = 10
VTS_BUFS = 10
EXP_SHIFT = -17.0

Exp = mybir.ActivationFunctionType.Exp
AX = mybir.AxisListType.X


def build_module(debug: bool = False) -> bass.Bass:
    nc = bacc.Bacc("TRN2", target_bir_lowering=False)

    x = nc.declare_dram_parameter("x", [C, HW], DT, isOutput=False)
    aux = nc.declare_dram_parameter("aux", [C, HW], DT, isOutput=False)
    # conv weights arrive pre-transposed from the host (numpy .T is free)
    WqT_d = nc.declare_dram_parameter("WqT_d", [C, CH], DT, isOutput=False)
    Wq_b = nc.declare_dram_parameter("Wq_b", [CH], DT, isOutput=False)
    WkT_d = nc.declare_dram_parameter("WkT_d", [C, CH], DT, isOutput=False)
    Wk_b = nc.declare_dram_parameter("Wk_b", [CH], DT, isOutput=False)
    WvT_d = nc.declare_dram_parameter("WvT_d", [C, CH], DT, isOutput=False)
    Wv_b = nc.declare_dram_parameter("Wv_b", [CH], DT, isOutput=False)
    WzT_d = nc.declare_dram_parameter("WzT_d", [CH, C], DT, isOutput=False)
    Wz_b = nc.declare_dram_parameter("Wz_b", [C], DT, isOutput=False)
    z = nc.declare_dram_parameter("z", [C, HW], DT, isOutput=True)
    dbg = {}
    if debug:
        for nm, shape in (("dq", [P, HW]), ("dk", [P, HW]), ("dvT", [P, HW]),
                          ("dy", [P, HW]), ("dexp0", [P, HW]), ("dsums", [P, NIT])):
            dbg[nm] = nc.declare_dram_parameter(nm, shape, DT, isOutput=True)

    with tile.TileContext(nc) as tc:
        with (
            tc.tile_pool(name="consts", bufs=1) as consts,
            tc.tile_pool(name="sing", bufs=1) as sing,
            tc.tile_pool(name="expp", bufs=EXP_BUFS) as expp,
            tc.tile_pool(name="vtsp", bufs=VTS_BUFS) as vtsp,
            tc.tile_pool(name="instream", bufs=6) as instream,
            tc.tile_pool(name="wload", bufs=1) as wload,
            tc.tile_pool(name="smalls", bufs=8) as smalls,
            tc.tile_pool(name="zst", bufs=3) as zst,
            tc.tile_pool(name="psK", bufs=2, space="PSUM") as psK,
            tc.tile_pool(name="psY", bufs=2, space="PSUM") as psY,
        ):
            # ---- head DMA stream (sync queue is FIFO: order = priority;
            #      the chain to the first exp is Wq,x0,qb -> Wk,aux0,kb) ----
            wts: dict[str, bass.AP] = {}

            def emit_w_dma(name, w_dram):
                wt = wload.tile([P, 2, P], DT, tag="wl" + name)
                for h in range(2):
                    nc.sync.dma_start(out=wt[:, h], in_=w_dram[h * P : (h + 1) * P, :])
                wts[name] = wt

            # x chunks stay resident: conv input now, +x residual at the tail
            xch: list = [None] * NJB

            xh_t: dict[int, tuple] = {}

            def emit_x_dma(cb: int, eng=None) -> None:
                js = cb * 512
                eng = eng or nc.sync
                x0 = sing.tile([P, 512], DT, name=f"x0_{cb}")
                eng.dma_start(out=x0, in_=x[0:P, js : js + 512])
                x1 = sing.tile([P, 512], DT, name=f"x1_{cb}")
                eng.dma_start(out=x1, in_=x[P:C, js : js + 512])
                xch[cb] = (x0, x1)

            def emit_x_cast(cb: int) -> None:
                x0, x1 = xch[cb]
                h0 = instream.tile([P, 512], F16, tag="xh", bufs=6)
                nc.vector.tensor_copy(h0, x0)
                h1 = instream.tile([P, 512], F16, tag="xh", bufs=6)
                nc.vector.tensor_copy(h1, x1)
                xh_t[cb] = (h0, h1)

            aux_t: dict[int, tuple] = {}

            ah_t: dict[int, tuple] = {}

            def emit_aux_dma(ac: int) -> None:
                a0 = instream.tile([P, 512], DT, tag="ains", bufs=6)
                nc.sync.dma_start(out=a0, in_=aux[0:P, ac * 512 : ac * 512 + 512])
                a1 = instream.tile([P, 512], DT, tag="ains", bufs=6)
                nc.sync.dma_start(out=a1, in_=aux[P:C, ac * 512 : ac * 512 + 512])
                aux_t[ac] = (a0, a1)

            def emit_aux_cast(ac: int) -> None:
                a0, a1 = aux_t[ac]
                h0 = instream.tile([P, 512], F16, tag="ah", bufs=6)
                nc.vector.tensor_copy(h0, a0)
                h1 = instream.tile([P, 512], F16, tag="ah", bufs=6)
                nc.vector.tensor_copy(h1, a1)
                ah_t[ac] = (h0, h1)

            emit_w_dma("q", WqT_d)
            emit_x_dma(0, eng=nc.scalar)
            emit_x_dma(1, eng=nc.scalar)
            emit_x_dma(2, eng=nc.scalar)
            qb_row = consts.tile([1, P], DT)
            nc.sync.dma_start(out=qb_row, in_=Wq_b[:].rearrange("(o p) -> o p", o=1))
            emit_w_dma("k", WkT_d)
            emit_aux_dma(0)
            kb_row = consts.tile([1, P], DT)
            nc.sync.dma_start(out=kb_row, in_=Wk_b[:].rearrange("(o p) -> o p", o=1))
            vb_row = consts.tile([1, P], DT)
            nc.sync.dma_start(out=vb_row, in_=Wv_b[:].rearrange("(o p) -> o p", o=1))
            emit_w_dma("v", WvT_d)
            wtz = wload.tile([P, C], DT, tag="wlz")
            nc.sync.dma_start(out=wtz, in_=WzT_d[:, :])
            for cb in range(3, NJB):
                emit_x_dma(cb)
            zb_row = consts.tile([1, C], DT)
            nc.sync.dma_start(out=zb_row, in_=Wz_b[:].rearrange("(o p) -> o p", o=1))
            emit_aux_dma(1)
            emit_aux_dma(2)

            # ---- consts (fp16 rows so the K=1 broadcast matmuls run
            #      1-pass; fp32 K=1 LOW_HIGH measured 13.5us for 5 tiles) ----
            ones512h = consts.tile([1, 512], F16)
            nc.vector.memset(ones512h, 1.0)
            eshift = consts.tile([P, 1], DT)
            nc.vector.memset(eshift, EXP_SHIFT)

            wts16: dict[str, bass.AP] = {}
            w16 = consts.tile([P, 2, P], F16, name="w16q")
            nc.vector.tensor_copy(w16, wts["q"])
            wts16["q"] = w16

            def row16(row_ap, width: int, name: str):
                r = consts.tile([1, width], F16, name=name)
                nc.vector.tensor_copy(r, row_ap)
                return r

            # bias broadcast tiles via K=1 matmuls (fp16 inputs, fp32 out).
            # per-partition ([128,1]-style) biases: bc[p, j] = bias[p]
            #   -> stationary = bias row, moving = ones row.
            # per-column (vT's c bias): bc[p, j] = bias[j]
            #   -> stationary = ones row, moving = bias row.
            def emit_bcast(stat_row, mov_row, width: int, name: str):
                ps = psY.tile([P, width], DT, tag="y")
                nc.tensor.matmul(ps, stat_row, mov_row[:, 0:width],
                                 start=True, stop=True)
                t = consts.tile([P, width], DT, name=name)
                nc.vector.tensor_copy(t, ps)
                return t

            qb_bc = emit_bcast(row16(qb_row, P, "qb16"), ones512h, 512, "qb_bc")
            for wname in ("k", "v"):
                w16 = consts.tile([P, 2, P], F16, name="w16" + wname)
                nc.vector.tensor_copy(w16, wts[wname])
                wts16[wname] = w16
            kb_bc = emit_bcast(row16(kb_row, P, "kb16"), ones512h, 512, "kb_bc")
            ones_row_h = consts.tile([1, P], F16)
            nc.vector.memset(ones_row_h, 1.0)
            vb_bc = emit_bcast(ones_row_h, row16(vb_row, P, "vb16"), P, "vb_bc")

            # late consts, emitted as fills inside group 1 (off the
            # critical path of the first exp)
            late: dict[str, bass.AP] = {}

            def emit_late_consts() -> None:
                late["WzT"] = consts.tile([P, 2, P], F16, name="WzT")
                nc.vector.tensor_copy(late["WzT"], wtz.rearrange("p (t q) -> p t q", t=2))
                late["zb_bc0"] = emit_bcast(
                    row16(zb_row[:, 0:P], P, "zb16_0"), ones512h, 512, "zb_bc0")
                late["zb_bc1"] = emit_bcast(
                    row16(zb_row[:, P:C], P, "zb16_1"), ones512h, 512, "zb_bc1")

            # x + zb precombine, in place on the idle GpSimd engine (x raw is
            # only needed by the q convs, all emitted in group 0): the z tail
            # then needs a single DVE add per psum flush.
            def emit_xzb(cb: int) -> None:
                x0, x1 = xch[cb]
                nc.gpsimd.tensor_add(x0, x0, late["zb_bc0"])
                nc.gpsimd.tensor_add(x1, x1, late["zb_bc1"])

            # ---- persistent operands ----
            q_sb = sing.tile([P, HW], F16)
            k_sb = sing.tile([P, HW], F16)
            vT_sb = sing.tile([P, HW], F16)   # 32 tiles of [i=128, c=128]
            y_sb = sing.tile([P, HW], F16)
            # softmax row sums: persistent so exp ACTIVATE carries no
            # pool-slot cross-engine dependency
            sums = sing.tile([P, NIT, len(EXP_CHUNKS)], DT)

            # ---- conv emitters (raw fp32, 2-pass PE; bias folded into the
            #      DVE flush) ----
            def emit_q(cb: int) -> None:
                js = cb * 512
                x0, x1 = xh_t[cb]
                qp = psY.tile([P, 512], DT, tag="y")
                nc.tensor.matmul(qp, wts16["q"][:, 0], x0, start=True, stop=False)
                nc.tensor.matmul(qp, wts16["q"][:, 1], x1, start=False, stop=True)
                nc.vector.tensor_add(q_sb[:, js : js + 512], qp, qb_bc)

            def emit_k(ac: int) -> None:
                js = ac * 512
                a0, a1 = ah_t[ac]
                kp = psY.tile([P, 512], DT, tag="y")
                nc.tensor.matmul(kp, wts16["k"][:, 0], a0, start=True, stop=False)
                nc.tensor.matmul(kp, wts16["k"][:, 1], a1, start=False, stop=True)
                nc.vector.tensor_add(k_sb[:, js : js + 512], kp, kb_bc)

            def emit_v(ac: int, half: int) -> None:
                # vT[i, c] for the 2 i-tiles in `half` of aux chunk ac
                a0, a1 = ah_t[ac]
                for ti in range(2):
                    t = half * 2 + ti
                    vp = psY.tile([P, P], DT, tag="y")
                    nc.tensor.matmul(vp, a0[:, t * P : (t + 1) * P], wts16["v"][:, 0],
                                     start=True, stop=False)
                    nc.tensor.matmul(vp, a1[:, t * P : (t + 1) * P], wts16["v"][:, 1],
                                     start=False, stop=True)
                    off = ac * 512 + t * P
                    nc.vector.tensor_add(vT_sb[:, off : off + P], vp, vb_bc)

            # ---- attention emitters ----
            exp_t: dict[int, bass.AP] = {}
            vts_t: dict[int, bass.AP] = {}

            def emit_ktq_chunk(it: int, ci: int) -> None:
                if ci == 0:
                    exp_t[it] = expp.tile([P, HW], F16, tag="exp", name="et")
                off, w = EXP_CHUNKS[ci]
                kt = psK.tile([P, w], DT, tag="kt")
                for s in range(w // 512):
                    nc.tensor.matmul(
                        kt[:, s * 512 : (s + 1) * 512],
                        k_sb[:, it * P : (it + 1) * P],
                        q_sb[:, off + s * 512 : off + (s + 1) * 512],
                        start=True, stop=True,
                    )
                nc.scalar.activation(
                    out=exp_t[it][:, off : off + w], in_=kt, func=Exp,
                    bias=eshift, scale=1.0,
                    accum_out=sums[:, it, ci : ci + 1],
                )

            def emit_fin(it: int) -> None:
                sv = smalls.tile([P, 1], DT, tag="sv")
                nc.vector.reduce_sum(sv, sums[:, it], axis=AX)
                rv = smalls.tile([P, 1], DT, tag="rv")
                nc.vector.reciprocal(rv, sv)
                vt = vtsp.tile([P, P], F16, tag="vts")
                nc.vector.tensor_scalar_mul(vt, vT_sb[:, it * P : (it + 1) * P], rv)
                vts_t[it] = vt

            def emit_y(g: int, jb: int) -> None:
                """y[:, jb] += vts.T @ exp over the 4 i-tiles of group g."""
                js = jb * 512
                yp = psY.tile([P, 512], DT, tag="y")
                for gi in range(G):
                    it = g * G + gi
                    nc.tensor.matmul(
                        yp, vts_t[it], exp_t[it][:, js : js + 512],
                        start=(gi == 0), stop=(gi == G - 1),
                    )
                if g == 0:
                    nc.vector.tensor_copy(y_sb[:, js : js + 512], yp)
                else:
                    nc.vector.tensor_add(
                        y_sb[:, js : js + 512], y_sb[:, js : js + 512], yp
                    )

            def emit_z(jb: int) -> None:
                # tail-only: ScalarE is idle after the last exp, so it does
                # the psum->sbuf flush; the +((x+zb)) add is split DVE/GpSimd
                js = jb * 512
                xzb = xch[jb]
                for h in range(2):
                    zp = psK.tile([P, 512], DT, tag="kt")
                    nc.tensor.matmul(zp, late["WzT"][:, h], y_sb[:, js : js + 512],
                                     start=True, stop=True)
                    zc = zst.tile([P, 512], DT, tag="zc")
                    nc.vector.tensor_add(zc, zp, xzb[h])
                    nc.sync.dma_start(out=z[h * P : (h + 1) * P, js : js + 512], in_=zc)

            # ================= schedule =================
            # warmup = group 0: q cols first (ktq moving operand), k/v for
            # group 0 just-in-time, interleaved with tile 0-3 ktq/exp; convs
            # for group 1 land inside group 0's stream.
            emit_x_cast(0)
            emit_q(0)
            emit_aux_cast(0)
            emit_k(0)
            emit_x_cast(1)
            emit_q(1)
            emit_x_cast(2)
            emit_q(2)
            emit_v(0, 0)
            emit_v(0, 1)
            emit_ktq_chunk(0, 0)
            emit_x_cast(3)
            emit_q(3)
            emit_x_cast(4)
            emit_q(4)
            emit_x_cast(5)
            emit_q(5)
            emit_ktq_chunk(0, 1)
            emit_x_cast(6)
            emit_q(6)
            emit_x_cast(7)
            emit_q(7)
            emit_ktq_chunk(0, 2)
            emit_fin(0)
            emit_ktq_chunk(1, 0)
            emit_aux_cast(1)
            emit_k(1)
            emit_ktq_chunk(1, 1)
            emit_v(1, 0)
            emit_ktq_chunk(1, 2)
            emit_fin(1)
            emit_v(1, 1)
            for it in (2, 3):
                for ci in range(3):
                    emit_ktq_chunk(it, ci)
                emit_fin(it)

            # steady state: group g runs its 12 ktq/exp chunks with fills:
            # y(g-1) x8, conv k/v for group g+1, aux prefetch for g+2.
            for g in range(1, NG):
                fills: list = []
                if g == 1:
                    fills.append(emit_late_consts)
                if 2 <= g <= 5:
                    for cb in (2 * g - 4, 2 * g - 3):
                        fills.append(lambda c=cb: emit_xzb(c))
                if g + 2 < NG:
                    fills.append(lambda a=g + 2: emit_aux_dma(a))
                fills.append(lambda gg=g: emit_y(gg - 1, 0))
                if g + 1 < NG:
                    fills.append(lambda a=g + 1: emit_aux_cast(a))
                    fills.append(lambda a=g + 1: emit_k(a))
                fills.append(lambda gg=g: emit_y(gg - 1, 1))
                if g + 1 < NG:
                    fills.append(lambda a=g + 1: emit_v(a, 0))
                fills.append(lambda gg=g: emit_y(gg - 1, 2))
                if g + 1 < NG:
                    fills.append(lambda a=g + 1: emit_v(a, 1))
                for jb in range(3, NJB):
                    fills.append(lambda gg=g, j=jb: emit_y(gg - 1, j))
                fi = 0
                nslots = G * 3
                nf = len(fills)
                acc = 0.0
                for t in range(G):
                    it = g * G + t
                    for ci in range(3):
                        emit_ktq_chunk(it, ci)
                        acc += nf / nslots
                        while fi < nf and fi < acc:
                            fills[fi]()
                            fi += 1
                    emit_fin(it)
                while fi < nf:
                    fills[fi]()
                    fi += 1

            # tail: y for the last group, z streamed per column block
            emit_y(NG - 1, 0)
            for jb in range(1, NJB):
                emit_y(NG - 1, jb)
                emit_z(jb - 1)
            emit_z(NJB - 1)
            if debug:
                def dump(dst, src_ap):
                    t = zst.tile([P, 512], DT, tag="zc")
                    nc.vector.tensor_copy(t, src_ap)
                    nc.sync.dma_start(out=dst, in_=t)
                for cb in range(NJB):
                    sl = slice(cb * 512, cb * 512 + 512)
                    dump(dbg["dq"][:, sl], q_sb[:, sl])
                    dump(dbg["dk"][:, sl], k_sb[:, sl])
                    dump(dbg["dvT"][:, sl], vT_sb[:, sl])
                    dump(dbg["dy"][:, sl], y_sb[:, sl])
                    dump(dbg["dexp0"][:, sl], exp_t[0][:, sl])
                sv2 = smalls.tile([P, NIT], DT, tag="dbg2")
                for it in range(NIT):
                    nc.vector.reduce_sum(sv2[:, it : it + 1], sums[:, it], axis=AX)
                nc.sync.dma_start(out=dbg["dsums"][:, :], in_=sv2)

    nc.compile()
    return nc


_NC = None


def _get_nc() -> bass.Bass:
    global _NC
    if _NC is None:
        _NC = build_module()
    return _NC


def _make_in_maps(inputs: dict[str, np.ndarray]) -> list[dict[str, np.ndarray]]:
    B = inputs["x"].shape[0]
    shared = {
        name: np.ascontiguousarray(np.asarray(inputs[name], dtype=np.float32))
        for name in ("Wq_b", "Wk_b", "Wv_b", "Wz_b")
    }
    for dev_name, host_name in (
        ("WqT_d", "Wq_w"), ("WkT_d", "Wk_w"), ("WvT_d", "Wv_w"), ("WzT_d", "Wz_w"),
    ):
        shared[dev_name] = np.ascontiguousarray(
            np.asarray(inputs[host_name], dtype=np.float32).T
        )
    in_maps = []
    for b in range(B):
        m = dict(shared)
        m["x"] = np.ascontiguousarray(
            np.asarray(inputs["x"][b], dtype=np.float32).reshape(C, HW)
        )
        m["aux"] = np.ascontiguousarray(
            np.asarray(inputs["aux"][b], dtype=np.float32).reshape(C, HW)
        )
        in_maps.append(m)
    return in_maps


def _install_ntff_hook_shim() -> None:
    """The agent image's antenv lacks axon_hooks; recreate it so
    run_bass_kernel_spmd(trace=True) can reach the libaxon NTFF profiler."""
    import types

    if "antenv.axon_hooks" in sys.modules:
        return
    import antenv

    mod = types.ModuleType("antenv.axon_hooks")
    state = {"hook": None}
    mod.set_axon_ntff_profile_hook = lambda h: state.__setitem__("hook", h)
    mod.get_axon_ntff_profile_hook = lambda: state["hook"]
    sys.modules["antenv.axon_hooks"] = mod
    antenv.axon_hooks = mod
    try:
        from trn_agent_boot.trn_boot import _ntff_profile_via_ctypes

        hook = _ntff_profile_via_ctypes("/opt/axon/libaxon_pjrt.so")
        if hook is not None:
            mod.set_axon_ntff_profile_hook(hook)
    except Exception as e:  # degrade to no tracing
        print(f"ntff hook unavailable: {e}", file=sys.stderr)


def run(inputs: dict[str, np.ndarray], trace: bool = False):
    """Run on the 8 NeuronCores; returns (output [8,256,64,64], BassKernelResults)."""
    from concourse.bass_utils import run_bass_kernel_spmd

    if trace:
        _install_ntff_hook_shim()
    nc = _get_nc()
    in_maps = _make_in_maps(inputs)
    res = run_bass_kernel_spmd(nc, in_maps, list(range(len(in_maps))), trace=trace)
    out = np.stack([r["z"].reshape(C, 64, 64) for r in res.results])
    return out.astype(np.float32), res


def kernel(**inputs: np.ndarray) -> np.ndarray:
    out, _ = run(inputs, trace=False)
    return out


if __name__ == "__main__":
    nc = build_module()
    print("module built ok")


# revision 12
# speedup vs baseline: 1.1964x; 1.1534x over previous
"""Trainium2 Bass kernel for nn_AttentionV2 (dense transformer attention block).

Reference computation (per batch element b):
    q  = Wq @ x_b  + qb          # [128, 4096]  (1x1 conv over channels)
    k  = Wk @ aux_b + kb         # [128, 4096]
    v  = Wv @ aux_b + vb         # [128, 4096]
    ktq[i, j] = sum_c k[c, i] * q[c, j]          # [4096, 4096]
    atten = softmax(ktq, axis=j)
    y[c, j] = sum_i v[c, i] * atten[i, j]        # [128, 4096]
    z = Wz @ y + zb + x_b        # [256, 4096]

Sharding: batch B=8 across the 8 cores (data parallel, weights replicated).
Each core runs the whole attention for its batch element; no collectives.

v2 design (constants measured on hw via a probe kernel):
  * PE streams 512-col matmuls at 2.4GHz (216ns) for 16-bit dtypes and
    fp32r with LDWEIGHTS fully hidden; raw fp32 runs 2-pass (429ns).  The
    two big matmuls (ktq, y) are 256 x 512-col each -> ~112us PE floor;
    ScalarE exp of all 16.8M logits is ~150-160us -> ACT is the bottleneck
    and the PE must never let it starve.
  * q/k/exp/vts/y all fp16 (flushes cast); convs run on raw fp32 inputs
    (2-pass PE) so no input casts exist at all (GpSimd casts measured
    3.5ns/elem - useless; DVE cycles are precious).
  * softmax is unnormalized (shift -17 keeps exp in fp16 range; max logit
    ~26) with the row-sum reciprocal folded into vts; row sums come from
    ACT accum_out (+182ns/chunk measured; a DVE fp16 reduce gets no 2x
    mode and would cost 4.4us/tile).
  * psum: 2 x [128,1536] rotating ktq chunk slots (6 banks) feed exp;
    2 x [128,512] slots (2 banks) shared by y accumulation, convs, the
    bias-broadcast builders and the z tail.
  * i-tiles in 8 groups of 4 (one 512-col aux chunk per group); y matmuls
    for group g run during g+1, interleaved between ktq chunks so the PE
    queue never head-of-line blocks on a psum-slot wait.
  * all biases are applied on DVE during psum->sbuf flushes; per-partition
    bias columns are built as [128,w] broadcast tiles by K=1 matmuls from
    contiguous [1,128] bias rows (no scatter DMAs, no ACT identity work).
  * x conv chunks stay resident in SBUF and are reused for the +x residual
    (saves 4MB of tail DMA).
"""

import sys

if "/opt/trn_rl_repo" not in sys.path:
    sys.path.insert(0, "/opt/trn_rl_repo")

import numpy as np

import concourse.bass as bass
import concourse.bacc as bacc
import concourse.mybir as mybir
import concourse.tile as tile

DT = mybir.dt.float32
F16 = mybir.dt.float16
P = 128          # partitions
C = 256          # input channels
CH = 128         # conv output channels (C//2)
HW = 4096        # 64*64 spatial
NJB = HW // 512  # 8 column blocks of 512
NIT = HW // P    # 32 i-tiles
G = 4            # i-tiles per group == i-tiles per 512-col aux chunk
NG = NIT // G    # 8 groups
EXP_CHUNKS = ((0, 1536), (1536, 1536), (3072, 1024))
# tile 0 starts with a mini chunk so the first exp fires as soon as the
# first x/aux chunks land (saves ~10us of ScalarE head idle)
EXP_CHUNKS_T0 = ((0, 512), (512, 1024), (1536, 1536), (3072, 1024))
EXP_BUFS = 10
VTS_BUFS = 10
EXP_SHIFT = -17.0

Exp = mybir.ActivationFunctionType.Exp
AX = mybir.AxisListType.X


def build_module(debug: bool = False) -> bass.Bass:
    nc = bacc.Bacc("TRN2", target_bir_lowering=False)

    x = nc.declare_dram_parameter("x", [C, HW], DT, isOutput=False)
    aux = nc.declare_dram_parameter("aux", [C, HW], DT, isOutput=False)
    # conv weights arrive pre-transposed from the host (numpy .T is free)
    WqT_d = nc.declare_dram_parameter("WqT_d", [C, CH], DT, isOutput=False)
    Wq_b = nc.declare_dram_parameter("Wq_b", [CH], DT, isOutput=False)
    WkT_d = nc.declare_dram_parameter("WkT_d", [C, CH], DT, isOutput=False)
    Wk_b = nc.declare_dram_parameter("Wk_b", [CH], DT, isOutput=False)
    WvT_d = nc.declare_dram_parameter("WvT_d", [C, CH], DT, isOutput=False)
    Wv_b = nc.declare_dram_parameter("Wv_b", [CH], DT, isOutput=False)
    WzT_d = nc.declare_dram_parameter("WzT_d", [CH, C], DT, isOutput=False)
    Wz_b = nc.declare_dram_parameter("Wz_b", [C], DT, isOutput=False)
    z = nc.declare_dram_parameter("z", [C, HW], DT, isOutput=True)
    dbg = {}
    if debug:
        for nm, shape in (("dq", [P, HW]), ("dk", [P, HW]), ("dvT", [P, HW]),
                          ("dy", [P, HW]), ("dexp0", [P, HW]), ("dsums", [P, NIT])):
            dbg[nm] = nc.declare_dram_parameter(nm, shape, DT, isOutput=True)

    with tile.TileContext(nc) as tc:
        with (
            tc.tile_pool(name="consts", bufs=1) as consts,
            tc.tile_pool(name="sing", bufs=1) as sing,
            tc.tile_pool(name="expp", bufs=EXP_BUFS) as expp,
            tc.tile_pool(name="vtsp", bufs=VTS_BUFS) as vtsp,
            tc.tile_pool(name="instream", bufs=6) as instream,
            tc.tile_pool(name="wload", bufs=1) as wload,
            tc.tile_pool(name="smalls", bufs=8) as smalls,
            tc.tile_pool(name="zst", bufs=3) as zst,
            tc.tile_pool(name="psK", bufs=2, space="PSUM") as psK,
            tc.tile_pool(name="psY", bufs=2, space="PSUM") as psY,
        ):
            # ---- head DMA stream (sync queue is FIFO: order = priority;
            #      the chain to the first exp is Wq,x0,qb -> Wk,aux0,kb) ----
            wts: dict[str, bass.AP] = {}

            def emit_w_dma(name, w_dram):
                wt = wload.tile([P, 2, P], DT, tag="wl" + name)
                for h in range(2):
                    nc.sync.dma_start(out=wt[:, h], in_=w_dram[h * P : (h + 1) * P, :])
                wts[name] = wt

            # x chunks stay resident: conv input now, +x residual at the tail
            xch: list = [None] * NJB

            xh_t: dict[int, tuple] = {}

            def emit_x_dma(cb: int, eng=None) -> None:
                js = cb * 512
                eng = eng or nc.sync
                x0 = sing.tile([P, 512], DT, name=f"x0_{cb}")
                eng.dma_start(out=x0, in_=x[0:P, js : js + 512])
                x1 = sing.tile([P, 512], DT, name=f"x1_{cb}")
                eng.dma_start(out=x1, in_=x[P:C, js : js + 512])
                xch[cb] = (x0, x1)

            def emit_x_cast(cb: int) -> None:
                x0, x1 = xch[cb]
                h0 = instream.tile([P, 512], F16, tag="xh", bufs=6)
                nc.vector.tensor_copy(h0, x0)
                h1 = instream.tile([P, 512], F16, tag="xh", bufs=6)
                nc.vector.tensor_copy(h1, x1)
                xh_t[cb] = (h0, h1)

            aux_t: dict[int, tuple] = {}

            ah_t: dict[int, tuple] = {}

            def emit_aux_dma(ac: int) -> None:
                a0 = instream.tile([P, 512], DT, tag="ains", bufs=6)
                nc.sync.dma_start(out=a0, in_=aux[0:P, ac * 512 : ac * 512 + 512])
                a1 = instream.tile([P, 512], DT, tag="ains", bufs=6)
                nc.sync.dma_start(out=a1, in_=aux[P:C, ac * 512 : ac * 512 + 512])
                aux_t[ac] = (a0, a1)

            def emit_aux_cast(ac: int) -> None:
                a0, a1 = aux_t[ac]
                h0 = instream.tile([P, 512], F16, tag="ah", bufs=6)
                nc.vector.tensor_copy(h0, a0)
                h1 = instream.tile([P, 512], F16, tag="ah", bufs=6)
                nc.vector.tensor_copy(h1, a1)
                ah_t[ac] = (h0, h1)

            emit_w_dma("q", WqT_d)
            emit_x_dma(0, eng=nc.scalar)
            emit_x_dma(1, eng=nc.scalar)
            emit_x_dma(2, eng=nc.scalar)
            qb_row = consts.tile([1, P], DT)
            nc.sync.dma_start(out=qb_row, in_=Wq_b[:].rearrange("(o p) -> o p", o=1))
            emit_w_dma("k", WkT_d)
            emit_aux_dma(0)
            kb_row = consts.tile([1, P], DT)
            nc.sync.dma_start(out=kb_row, in_=Wk_b[:].rearrange("(o p) -> o p", o=1))
            vb_row = consts.tile([1, P], DT)
            nc.sync.dma_start(out=vb_row, in_=Wv_b[:].rearrange("(o p) -> o p", o=1))
            emit_w_dma("v", WvT_d)
            wtz = wload.tile([P, C], DT, tag="wlz")
            nc.sync.dma_start(out=wtz, in_=WzT_d[:, :])
            for cb in range(3, NJB):
                emit_x_dma(cb)
            zb_row = consts.tile([1, C], DT)
            nc.sync.dma_start(out=zb_row, in_=Wz_b[:].rearrange("(o p) -> o p", o=1))
            emit_aux_dma(1)
            emit_aux_dma(2)

            # ---- consts (fp16 rows so the K=1 broadcast matmuls run
            #      1-pass; fp32 K=1 LOW_HIGH measured 13.5us for 5 tiles) ----
            ones512h = consts.tile([1, 512], F16)
            nc.vector.memset(ones512h, 1.0)
            eshift = consts.tile([P, 1], DT)
            nc.vector.memset(eshift, EXP_SHIFT)

            wts16: dict[str, bass.AP] = {}
            w16 = consts.tile([P, 2, P], F16, name="w16q")
            nc.vector.tensor_copy(w16, wts["q"])
            wts16["q"] = w16

            def row16(row_ap, width: int, name: str):
                r = consts.tile([1, width], F16, name=name)
                nc.vector.tensor_copy(r, row_ap)
                return r

            # bias broadcast tiles via K=1 matmuls (fp16 inputs, fp32 out).
            # per-partition ([128,1]-style) biases: bc[p, j] = bias[p]
            #   -> stationary = bias row, moving = ones row.
            # per-column (vT's c bias): bc[p, j] = bias[j]
            #   -> stationary = ones row, moving = bias row.
            def emit_bcast(stat_row, mov_row, width: int, name: str):
                ps = psY.tile([P, width], DT, tag="y")
                nc.tensor.matmul(ps, stat_row, mov_row[:, 0:width],
                                 start=True, stop=True)
                t = consts.tile([P, width], DT, name=name)
                nc.vector.tensor_copy(t, ps)
                return t

            qb_bc = emit_bcast(row16(qb_row, P, "qb16"), ones512h, 512, "qb_bc")
            for wname in ("k", "v"):
                w16 = consts.tile([P, 2, P], F16, name="w16" + wname)
                nc.vector.tensor_copy(w16, wts[wname])
                wts16[wname] = w16
            kb_bc = emit_bcast(row16(kb_row, P, "kb16"), ones512h, 512, "kb_bc")
            ones_row_h = consts.tile([1, P], F16)
            nc.vector.memset(ones_row_h, 1.0)
            vb_bc = emit_bcast(ones_row_h, row16(vb_row, P, "vb16"), P, "vb_bc")

            # late consts, emitted as fills inside group 1 (off the
            # critical path of the first exp)
            late: dict[str, bass.AP] = {}

            def emit_late_consts() -> None:
                late["WzT"] = consts.tile([P, 2, P], F16, name="WzT")
                nc.vector.tensor_copy(late["WzT"], wtz.rearrange("p (t q) -> p t q", t=2))
                late["zb_bc0"] = emit_bcast(
                    row16(zb_row[:, 0:P], P, "zb16_0"), ones512h, 512, "zb_bc0")
                late["zb_bc1"] = emit_bcast(
                    row16(zb_row[:, P:C], P, "zb16_1"), ones512h, 512, "zb_bc1")

            # x + zb precombine, in place on the idle GpSimd engine (x raw is
            # only needed by the q convs, all emitted in group 0): the z tail
            # then needs a single DVE add per psum flush.
            def emit_xzb(cb: int) -> None:
                x0, x1 = xch[cb]
                nc.gpsimd.tensor_add(x0, x0, late["zb_bc0"])
                nc.gpsimd.tensor_add(x1, x1, late["zb_bc1"])

            # ---- persistent operands ----
            q_sb = sing.tile([P, HW], F16)
            k_sb = sing.tile([P, HW], F16)
            vT_sb = sing.tile([P, HW], F16)   # 32 tiles of [i=128, c=128]
            y_sb = sing.tile([P, HW], F16)
            # softmax row sums: persistent so exp ACTIVATE carries no
            # pool-slot cross-engine dependency
            sums = sing.tile([P, NIT, len(EXP_CHUNKS_T0)], DT)

            # ---- conv emitters (raw fp32, 2-pass PE; bias folded into the
            #      DVE flush) ----
            def emit_q(cb: int) -> None:
                js = cb * 512
                x0, x1 = xh_t[cb]
                qp = psY.tile([P, 512], DT, tag="y")
                nc.tensor.matmul(qp, wts16["q"][:, 0], x0, start=True, stop=False)
                nc.tensor.matmul(qp, wts16["q"][:, 1], x1, start=False, stop=True)
                nc.vector.tensor_add(q_sb[:, js : js + 512], qp, qb_bc)

            def emit_k(ac: int) -> None:
                js = ac * 512
                a0, a1 = ah_t[ac]
                kp = psY.tile([P, 512], DT, tag="y")
                nc.tensor.matmul(kp, wts16["k"][:, 0], a0, start=True, stop=False)
                nc.tensor.matmul(kp, wts16["k"][:, 1], a1, start=False, stop=True)
                nc.vector.tensor_add(k_sb[:, js : js + 512], kp, kb_bc)

            def emit_v(ac: int, half: int) -> None:
                # vT[i, c] for the 2 i-tiles in `half` of aux chunk ac
                a0, a1 = ah_t[ac]
                for ti in range(2):
                    t = half * 2 + ti
                    vp = psY.tile([P, P], DT, tag="y")
                    nc.tensor.matmul(vp, a0[:, t * P : (t + 1) * P], wts16["v"][:, 0],
                                     start=True, stop=False)
                    nc.tensor.matmul(vp, a1[:, t * P : (t + 1) * P], wts16["v"][:, 1],
                                     start=False, stop=True)
                    off = ac * 512 + t * P
                    nc.vector.tensor_add(vT_sb[:, off : off + P], vp, vb_bc)

            # ---- attention emitters ----
            exp_t: dict[int, bass.AP] = {}
            vts_t: dict[int, bass.AP] = {}

            def chunks_of(it: int):
                return EXP_CHUNKS_T0 if it == 0 else EXP_CHUNKS

            def emit_ktq_chunk(it: int, ci: int) -> None:
                if ci == 0:
                    exp_t[it] = expp.tile([P, HW], F16, tag="exp", name="et")
                off, w = chunks_of(it)[ci]
                kt = psK.tile([P, w], DT, tag="kt")
                for s in range(w // 512):
                    nc.tensor.matmul(
                        kt[:, s * 512 : (s + 1) * 512],
                        k_sb[:, it * P : (it + 1) * P],
                        q_sb[:, off + s * 512 : off + (s + 1) * 512],
                        start=True, stop=True,
                    )
                nc.scalar.activation(
                    out=exp_t[it][:, off : off + w], in_=kt, func=Exp,
                    bias=eshift, scale=1.0,
                    accum_out=sums[:, it, ci : ci + 1],
                )

            def emit_fin(it: int) -> None:
                sv = smalls.tile([P, 1], DT, tag="sv")
                nc.vector.reduce_sum(sv, sums[:, it, 0 : len(chunks_of(it))], axis=AX)
                rv = smalls.tile([P, 1], DT, tag="rv")
                nc.vector.reciprocal(rv, sv)
                vt = vtsp.tile([P, P], F16, tag="vts")
                nc.vector.tensor_scalar_mul(vt, vT_sb[:, it * P : (it + 1) * P], rv)
                vts_t[it] = vt

            def emit_y(g: int, jb: int) -> None:
                """y[:, jb] += vts.T @ exp over the 4 i-tiles of group g."""
                js = jb * 512
                yp = psY.tile([P, 512], DT, tag="y")
                for gi in range(G):
                    it = g * G + gi
                    nc.tensor.matmul(
                        yp, vts_t[it], exp_t[it][:, js : js + 512],
                        start=(gi == 0), stop=(gi == G - 1),
                    )
                if g == 0:
                    nc.vector.tensor_copy(y_sb[:, js : js + 512], yp)
                else:
                    nc.vector.tensor_add(
                        y_sb[:, js : js + 512], y_sb[:, js : js + 512], yp
                    )

            def emit_z(jb: int) -> None:
                # tail-only: ScalarE is idle after the last exp, so it does
                # the psum->sbuf flush; the +((x+zb)) add is split DVE/GpSimd
                js = jb * 512
                xzb = xch[jb]
                for h in range(2):
                    zp = psK.tile([P, 512], DT, tag="kt")
                    nc.tensor.matmul(zp, late["WzT"][:, h], y_sb[:, js : js + 512],
                                     start=True, stop=True)
                    zc = zst.tile([P, 512], DT, tag="zc")
                    nc.vector.tensor_add(zc, zp, xzb[h])
                    nc.sync.dma_start(out=z[h * P : (h + 1) * P, js : js + 512], in_=zc)

            # ================= schedule =================
            # warmup = group 0: q cols first (ktq moving operand), k/v for
            # group 0 just-in-time, interleaved with tile 0-3 ktq/exp; convs
            # for group 1 land inside group 0's stream.
            emit_x_cast(0)
            emit_q(0)
            emit_aux_cast(0)
            emit_k(0)
            emit_ktq_chunk(0, 0)
            emit_x_cast(1)
            emit_q(1)
            emit_x_cast(2)
            emit_q(2)
            emit_ktq_chunk(0, 1)
            emit_v(0, 0)
            emit_v(0, 1)
            emit_x_cast(3)
            emit_q(3)
            emit_x_cast(4)
            emit_q(4)
            emit_x_cast(5)
            emit_q(5)
            emit_ktq_chunk(0, 2)
            emit_x_cast(6)
            emit_q(6)
            emit_x_cast(7)
            emit_q(7)
            emit_ktq_chunk(0, 3)
            emit_fin(0)
            emit_ktq_chunk(1, 0)
            emit_aux_cast(1)
            emit_k(1)
            emit_ktq_chunk(1, 1)
            emit_v(1, 0)
            emit_ktq_chunk(1, 2)
            emit_fin(1)
            emit_v(1, 1)
            for it in (2, 3):
                for ci in range(3):
                    emit_ktq_chunk(it, ci)
                emit_fin(it)

            # steady state: group g runs its 12 ktq/exp chunks with fills:
            # y(g-1) x8, conv k/v for group g+1, aux prefetch for g+2.
            for g in range(1, NG):
                fills: list = []
                if g == 1:
                    fills.append(emit_late_consts)
                if 2 <= g <= 5:
                    for cb in (2 * g - 4, 2 * g - 3):
                        fills.append(lambda c=cb: emit_xzb(c))
                if g + 2 < NG:
                    fills.append(lambda a=g + 2: emit_aux_dma(a))
                fills.append(lambda gg=g: emit_y(gg - 1, 0))
                if g + 1 < NG:
                    fills.append(lambda a=g + 1: emit_aux_cast(a))
                    fills.append(lambda a=g + 1: emit_k(a))
                fills.append(lambda gg=g: emit_y(gg - 1, 1))
                if g + 1 < NG:
                    fills.append(lambda a=g + 1: emit_v(a, 0))
                fills.append(lambda gg=g: emit_y(gg - 1, 2))
                if g + 1 < NG:
                    fills.append(lambda a=g + 1: emit_v(a, 1))
                for jb in range(3, NJB):
                    fills.append(lambda gg=g, j=jb: emit_y(gg - 1, j))
                fi = 0
                nslots = G * 3
                nf = len(fills)
                acc = 0.0
                for t in range(G):
                    it = g * G + t
                    for ci in range(3):
                        emit_ktq_chunk(it, ci)
                        acc += nf / nslots
                        while fi < nf and fi < acc:
                            fills[fi]()
                            fi += 1
                    emit_fin(it)
                while fi < nf:
                    fills[fi]()
                    fi += 1

            # tail: y for the last group, z streamed per column block
            emit_y(NG - 1, 0)
            for jb in range(1, NJB):
                emit_y(NG - 1, jb)
                emit_z(jb - 1)
            emit_z(NJB - 1)
            if debug:
                def dump(dst, src_ap):
                    t = zst.tile([P, 512], DT, tag="zc")
                    nc.vector.tensor_copy(t, src_ap)
                    nc.sync.dma_start(out=dst, in_=t)
                for cb in range(NJB):
                    sl = slice(cb * 512, cb * 512 + 512)
                    dump(dbg["dq"][:, sl], q_sb[:, sl])
                    dump(dbg["dk"][:, sl], k_sb[:, sl])
                    dump(dbg["dvT"][:, sl], vT_sb[:, sl])
                    dump(dbg["dy"][:, sl], y_sb[:, sl])
                    dump(dbg["dexp0"][:, sl], exp_t[0][:, sl])
                sv2 = smalls.tile([P, NIT], DT, tag="dbg2")
                for it in range(NIT):
                    nc.vector.reduce_sum(sv2[:, it : it + 1], sums[:, it], axis=AX)
                nc.sync.dma_start(out=dbg["dsums"][:, :], in_=sv2)

    nc.compile()
    return nc


_NC = None


def _get_nc() -> bass.Bass:
    global _NC
    if _NC is None:
        _NC = build_module()
    return _NC


def _make_in_maps(inputs: dict[str, np.ndarray]) -> list[dict[str, np.ndarray]]:
    B = inputs["x"].shape[0]
    shared = {
        name: np.ascontiguousarray(np.asarray(inputs[name], dtype=np.float32))
        for name in ("Wq_b", "Wk_b", "Wv_b", "Wz_b")
    }
    for dev_name, host_name in (
        ("WqT_d", "Wq_w"), ("WkT_d", "Wk_w"), ("WvT_d", "Wv_w"), ("WzT_d", "Wz_w"),
    ):
        shared[dev_name] = np.ascontiguousarray(
            np.asarray(inputs[host_name], dtype=np.float32).T
        )
    in_maps = []
    for b in range(B):
        m = dict(shared)
        m["x"] = np.ascontiguousarray(
            np.asarray(inputs["x"][b], dtype=np.float32).reshape(C, HW)
        )
        m["aux"] = np.ascontiguousarray(
            np.asarray(inputs["aux"][b], dtype=np.float32).reshape(C, HW)
        )
        in_maps.append(m)
    return in_maps


def _install_ntff_hook_shim() -> None:
    """The agent image's antenv lacks axon_hooks; recreate it so
    run_bass_kernel_spmd(trace=True) can reach the libaxon NTFF profiler."""
    import types

    if "antenv.axon_hooks" in sys.modules:
        return
    import antenv

    mod = types.ModuleType("antenv.axon_hooks")
    state = {"hook": None}
    mod.set_axon_ntff_profile_hook = lambda h: state.__setitem__("hook", h)
    mod.get_axon_ntff_profile_hook = lambda: state["hook"]
    sys.modules["antenv.axon_hooks"] = mod
    antenv.axon_hooks = mod
    try:
        from trn_agent_boot.trn_boot import _ntff_profile_via_ctypes

        hook = _ntff_profile_via_ctypes("/opt/axon/libaxon_pjrt.so")
        if hook is not None:
            mod.set_axon_ntff_profile_hook(hook)
    except Exception as e:  # degrade to no tracing
        print(f"ntff hook unavailable: {e}", file=sys.stderr)


def run(inputs: dict[str, np.ndarray], trace: bool = False):
    """Run on the 8 NeuronCores; returns (output [8,256,64,64], BassKernelResults)."""
    from concourse.bass_utils import run_bass_kernel_spmd

    if trace:
        _install_ntff_hook_shim()
    nc = _get_nc()
    in_maps = _make_in_maps(inputs)
    res = run_bass_kernel_spmd(nc, in_maps, list(range(len(in_maps))), trace=trace)
    out = np.stack([r["z"].reshape(C, 64, 64) for r in res.results])
    return out.astype(np.float32), res


def kernel(**inputs: np.ndarray) -> np.ndarray:
    out, _ = run(inputs, trace=False)
    return out


if __name__ == "__main__":
    nc = build_module()
    print("module built ok")


# revision 13
# speedup vs baseline: 1.2112x; 1.0124x over previous
"""Trainium2 Bass kernel for nn_AttentionV2 (dense transformer attention block).

Reference computation (per batch element b):
    q  = Wq @ x_b  + qb          # [128, 4096]  (1x1 conv over channels)
    k  = Wk @ aux_b + kb         # [128, 4096]
    v  = Wv @ aux_b + vb         # [128, 4096]
    ktq[i, j] = sum_c k[c, i] * q[c, j]          # [4096, 4096]
    atten = softmax(ktq, axis=j)
    y[c, j] = sum_i v[c, i] * atten[i, j]        # [128, 4096]
    z = Wz @ y + zb + x_b        # [256, 4096]

Sharding: batch B=8 across the 8 cores (data parallel, weights replicated).
Each core runs the whole attention for its batch element; no collectives.

v2 design (constants measured on hw via a probe kernel):
  * PE streams 512-col matmuls at 2.4GHz (216ns) for 16-bit dtypes and
    fp32r with LDWEIGHTS fully hidden; raw fp32 runs 2-pass (429ns).  The
    two big matmuls (ktq, y) are 256 x 512-col each -> ~112us PE floor;
    ScalarE exp of all 16.8M logits is ~150-160us -> ACT is the bottleneck
    and the PE must never let it starve.
  * q/k/exp/vts/y all fp16 (flushes cast); convs run on raw fp32 inputs
    (2-pass PE) so no input casts exist at all (GpSimd casts measured
    3.5ns/elem - useless; DVE cycles are precious).
  * softmax is unnormalized (shift -17 keeps exp in fp16 range; max logit
    ~26) with the row-sum reciprocal folded into vts; row sums come from
    ACT accum_out (+182ns/chunk measured; a DVE fp16 reduce gets no 2x
    mode and would cost 4.4us/tile).
  * psum: 2 x [128,1536] rotating ktq chunk slots (6 banks) feed exp;
    2 x [128,512] slots (2 banks) shared by y accumulation, convs, the
    bias-broadcast builders and the z tail.
  * i-tiles in 8 groups of 4 (one 512-col aux chunk per group); y matmuls
    for group g run during g+1, interleaved between ktq chunks so the PE
    queue never head-of-line blocks on a psum-slot wait.
  * all biases are applied on DVE during psum->sbuf flushes; per-partition
    bias columns are built as [128,w] broadcast tiles by K=1 matmuls from
    contiguous [1,128] bias rows (no scatter DMAs, no ACT identity work).
  * x conv chunks stay resident in SBUF and are reused for the +x residual
    (saves 4MB of tail DMA).
"""

import sys

if "/opt/trn_rl_repo" not in sys.path:
    sys.path.insert(0, "/opt/trn_rl_repo")

import numpy as np

import concourse.bass as bass
import concourse.bacc as bacc
import concourse.mybir as mybir
import concourse.tile as tile

DT = mybir.dt.float32
F16 = mybir.dt.float16
P = 128          # partitions
C = 256          # input channels
CH = 128         # conv output channels (C//2)
HW = 4096        # 64*64 spatial
NJB = HW // 512  # 8 column blocks of 512
NIT = HW // P    # 32 i-tiles
G = 4            # i-tiles per group == i-tiles per 512-col aux chunk
NG = NIT // G    # 8 groups
EXP_CHUNKS = ((0, 1536), (1536, 1536), (3072, 1024))
# tile 0 starts with a mini chunk so the first exp fires as soon as the
# first x/aux chunks land (saves ~10us of ScalarE head idle)
EXP_CHUNKS_T0 = ((0, 512), (512, 1024), (1536, 1536), (3072, 1024))
EXP_BUFS = 11
VTS_BUFS = 10
EXP_SHIFT = -17.0

Exp = mybir.ActivationFunctionType.Exp
AX = mybir.AxisListType.X


def build_module(debug: bool = False) -> bass.Bass:
    nc = bacc.Bacc("TRN2", target_bir_lowering=False)

    x = nc.declare_dram_parameter("x", [C, HW], DT, isOutput=False)
    aux = nc.declare_dram_parameter("aux", [C, HW], DT, isOutput=False)
    # conv weights arrive pre-transposed from the host (numpy .T is free)
    WqT_d = nc.declare_dram_parameter("WqT_d", [C, CH], DT, isOutput=False)
    Wq_b = nc.declare_dram_parameter("Wq_b", [CH], DT, isOutput=False)
    WkT_d = nc.declare_dram_parameter("WkT_d", [C, CH], DT, isOutput=False)
    Wk_b = nc.declare_dram_parameter("Wk_b", [CH], DT, isOutput=False)
    WvT_d = nc.declare_dram_parameter("WvT_d", [C, CH], DT, isOutput=False)
    Wv_b = nc.declare_dram_parameter("Wv_b", [CH], DT, isOutput=False)
    WzT_d = nc.declare_dram_parameter("WzT_d", [CH, C], DT, isOutput=False)
    Wz_b = nc.declare_dram_parameter("Wz_b", [C], DT, isOutput=False)
    z = nc.declare_dram_parameter("z", [C, HW], DT, isOutput=True)
    dbg = {}
    if debug:
        for nm, shape in (("dq", [P, HW]), ("dk", [P, HW]), ("dvT", [P, HW]),
                          ("dy", [P, HW]), ("dexp0", [P, HW]), ("dsums", [P, NIT])):
            dbg[nm] = nc.declare_dram_parameter(nm, shape, DT, isOutput=True)

    with tile.TileContext(nc) as tc:
        with (
            tc.tile_pool(name="consts", bufs=1) as consts,
            tc.tile_pool(name="sing", bufs=1) as sing,
            tc.tile_pool(name="expp", bufs=EXP_BUFS) as expp,
            tc.tile_pool(name="vtsp", bufs=VTS_BUFS) as vtsp,
            tc.tile_pool(name="instream", bufs=6) as instream,
            tc.tile_pool(name="wload", bufs=1) as wload,
            tc.tile_pool(name="smalls", bufs=8) as smalls,
            tc.tile_pool(name="zst", bufs=3) as zst,
            tc.tile_pool(name="psK", bufs=2, space="PSUM") as psK,
            tc.tile_pool(name="psY", bufs=2, space="PSUM") as psY,
        ):
            # ---- head DMA stream (sync queue is FIFO: order = priority;
            #      the chain to the first exp is Wq,x0,qb -> Wk,aux0,kb) ----
            wts: dict[str, bass.AP] = {}

            def emit_w_dma(name, w_dram):
                wt = wload.tile([P, 2, P], DT, tag="wl" + name)
                for h in range(2):
                    nc.sync.dma_start(out=wt[:, h], in_=w_dram[h * P : (h + 1) * P, :])
                wts[name] = wt

            # x chunks stay resident: conv input now, +x residual at the tail
            xch: list = [None] * NJB

            xh_t: dict[int, tuple] = {}

            def emit_x_dma(cb: int, eng=None) -> None:
                js = cb * 512
                eng = eng or nc.sync
                x0 = sing.tile([P, 512], DT, name=f"x0_{cb}")
                eng.dma_start(out=x0, in_=x[0:P, js : js + 512])
                x1 = sing.tile([P, 512], DT, name=f"x1_{cb}")
                eng.dma_start(out=x1, in_=x[P:C, js : js + 512])
                xch[cb] = (x0, x1)

            def emit_x_cast(cb: int) -> None:
                x0, x1 = xch[cb]
                h0 = instream.tile([P, 512], F16, tag="xh", bufs=6)
                nc.vector.tensor_copy(h0, x0)
                h1 = instream.tile([P, 512], F16, tag="xh", bufs=6)
                nc.vector.tensor_copy(h1, x1)
                xh_t[cb] = (h0, h1)

            aux_t: dict[int, tuple] = {}

            ah_t: dict[int, tuple] = {}

            def emit_aux_dma(ac: int) -> None:
                a0 = instream.tile([P, 512], DT, tag="ains", bufs=6)
                nc.sync.dma_start(out=a0, in_=aux[0:P, ac * 512 : ac * 512 + 512])
                a1 = instream.tile([P, 512], DT, tag="ains", bufs=6)
                nc.sync.dma_start(out=a1, in_=aux[P:C, ac * 512 : ac * 512 + 512])
                aux_t[ac] = (a0, a1)

            def emit_aux_cast(ac: int) -> None:
                a0, a1 = aux_t[ac]
                h0 = instream.tile([P, 512], F16, tag="ah", bufs=6)
                nc.vector.tensor_copy(h0, a0)
                h1 = instream.tile([P, 512], F16, tag="ah", bufs=6)
                nc.vector.tensor_copy(h1, a1)
                ah_t[ac] = (h0, h1)

            emit_x_dma(0, eng=nc.scalar)
            emit_w_dma("q", WqT_d)
            emit_aux_dma(0)
            emit_x_dma(1, eng=nc.scalar)
            emit_x_dma(2, eng=nc.scalar)
            qb_row = consts.tile([1, P], DT)
            nc.sync.dma_start(out=qb_row, in_=Wq_b[:].rearrange("(o p) -> o p", o=1))
            emit_w_dma("k", WkT_d)
            kb_row = consts.tile([1, P], DT)
            nc.sync.dma_start(out=kb_row, in_=Wk_b[:].rearrange("(o p) -> o p", o=1))
            vb_row = consts.tile([1, P], DT)
            nc.sync.dma_start(out=vb_row, in_=Wv_b[:].rearrange("(o p) -> o p", o=1))
            emit_w_dma("v", WvT_d)
            wtz = wload.tile([P, C], DT, tag="wlz")
            nc.sync.dma_start(out=wtz, in_=WzT_d[:, :])
            for cb in range(3, NJB):
                emit_x_dma(cb)
            zb_row = consts.tile([1, C], DT)
            nc.sync.dma_start(out=zb_row, in_=Wz_b[:].rearrange("(o p) -> o p", o=1))
            emit_aux_dma(1)
            emit_aux_dma(2)

            # ---- consts (fp16 rows so the K=1 broadcast matmuls run
            #      1-pass; fp32 K=1 LOW_HIGH measured 13.5us for 5 tiles) ----
            ones512h = consts.tile([1, 512], F16)
            nc.vector.memset(ones512h, 1.0)
            eshift = consts.tile([P, 1], DT)
            nc.vector.memset(eshift, EXP_SHIFT)

            wts16: dict[str, bass.AP] = {}
            w16 = consts.tile([P, 2, P], F16, name="w16q")
            nc.vector.tensor_copy(w16, wts["q"])
            wts16["q"] = w16

            def row16(row_ap, width: int, name: str):
                r = consts.tile([1, width], F16, name=name)
                nc.vector.tensor_copy(r, row_ap)
                return r

            # bias broadcast tiles via K=1 matmuls (fp16 inputs, fp32 out).
            # per-partition ([128,1]-style) biases: bc[p, j] = bias[p]
            #   -> stationary = bias row, moving = ones row.
            # per-column (vT's c bias): bc[p, j] = bias[j]
            #   -> stationary = ones row, moving = bias row.
            def emit_bcast(stat_row, mov_row, width: int, name: str):
                ps = psY.tile([P, width], DT, tag="y")
                nc.tensor.matmul(ps, stat_row, mov_row[:, 0:width],
                                 start=True, stop=True)
                t = consts.tile([P, width], DT, name=name)
                nc.vector.tensor_copy(t, ps)
                return t

            qb_bc = emit_bcast(row16(qb_row, P, "qb16"), ones512h, 512, "qb_bc")
            for wname in ("k", "v"):
                w16 = consts.tile([P, 2, P], F16, name="w16" + wname)
                nc.vector.tensor_copy(w16, wts[wname])
                wts16[wname] = w16
            kb_bc = emit_bcast(row16(kb_row, P, "kb16"), ones512h, 512, "kb_bc")
            ones_row_h = consts.tile([1, P], F16)
            nc.vector.memset(ones_row_h, 1.0)
            vb_bc = emit_bcast(ones_row_h, row16(vb_row, P, "vb16"), P, "vb_bc")

            # late consts, emitted as fills inside group 1 (off the
            # critical path of the first exp)
            late: dict[str, bass.AP] = {}

            def emit_late_consts() -> None:
                late["WzT"] = consts.tile([P, 2, P], F16, name="WzT")
                nc.vector.tensor_copy(late["WzT"], wtz.rearrange("p (t q) -> p t q", t=2))
                late["zb_bc0"] = emit_bcast(
                    row16(zb_row[:, 0:P], P, "zb16_0"), ones512h, 512, "zb_bc0")
                late["zb_bc1"] = emit_bcast(
                    row16(zb_row[:, P:C], P, "zb16_1"), ones512h, 512, "zb_bc1")

            # x + zb precombine, in place on the idle GpSimd engine (x raw is
            # only needed by the q convs, all emitted in group 0): the z tail
            # then needs a single DVE add per psum flush.
            def emit_xzb(cb: int) -> None:
                x0, x1 = xch[cb]
                nc.gpsimd.tensor_add(x0, x0, late["zb_bc0"])
                nc.gpsimd.tensor_add(x1, x1, late["zb_bc1"])

            # ---- persistent operands ----
            q_sb = sing.tile([P, HW], F16)
            k_sb = sing.tile([P, HW], F16)
            vT_sb = sing.tile([P, HW], F16)   # 32 tiles of [i=128, c=128]
            y_sb = sing.tile([P, HW], F16)
            # softmax row sums: persistent so exp ACTIVATE carries no
            # pool-slot cross-engine dependency
            sums = sing.tile([P, NIT, len(EXP_CHUNKS_T0)], DT)

            # ---- conv emitters (raw fp32, 2-pass PE; bias folded into the
            #      DVE flush) ----
            def emit_q(cb: int) -> None:
                js = cb * 512
                x0, x1 = xh_t[cb]
                qp = psY.tile([P, 512], DT, tag="y")
                nc.tensor.matmul(qp, wts16["q"][:, 0], x0, start=True, stop=False)
                nc.tensor.matmul(qp, wts16["q"][:, 1], x1, start=False, stop=True)
                nc.vector.tensor_add(q_sb[:, js : js + 512], qp, qb_bc)

            def emit_k(ac: int) -> None:
                js = ac * 512
                a0, a1 = ah_t[ac]
                kp = psY.tile([P, 512], DT, tag="y")
                nc.tensor.matmul(kp, wts16["k"][:, 0], a0, start=True, stop=False)
                nc.tensor.matmul(kp, wts16["k"][:, 1], a1, start=False, stop=True)
                nc.vector.tensor_add(k_sb[:, js : js + 512], kp, kb_bc)

            def emit_v(ac: int, half: int) -> None:
                # vT[i, c] for the 2 i-tiles in `half` of aux chunk ac
                a0, a1 = ah_t[ac]
                for ti in range(2):
                    t = half * 2 + ti
                    vp = psY.tile([P, P], DT, tag="y")
                    nc.tensor.matmul(vp, a0[:, t * P : (t + 1) * P], wts16["v"][:, 0],
                                     start=True, stop=False)
                    nc.tensor.matmul(vp, a1[:, t * P : (t + 1) * P], wts16["v"][:, 1],
                                     start=False, stop=True)
                    off = ac * 512 + t * P
                    nc.vector.tensor_add(vT_sb[:, off : off + P], vp, vb_bc)

            # ---- attention emitters ----
            exp_t: dict[int, bass.AP] = {}
            vts_t: dict[int, bass.AP] = {}

            def chunks_of(it: int):
                return EXP_CHUNKS_T0 if it == 0 else EXP_CHUNKS

            def emit_ktq_chunk(it: int, ci: int) -> None:
                if ci == 0:
                    exp_t[it] = expp.tile([P, HW], F16, tag="exp", name="et")
                off, w = chunks_of(it)[ci]
                kt = psK.tile([P, w], DT, tag="kt")
                for s in range(w // 512):
                    nc.tensor.matmul(
                        kt[:, s * 512 : (s + 1) * 512],
                        k_sb[:, it * P : (it + 1) * P],
                        q_sb[:, off + s * 512 : off + (s + 1) * 512],
                        start=True, stop=True,
                    )
                nc.scalar.activation(
                    out=exp_t[it][:, off : off + w], in_=kt, func=Exp,
                    bias=eshift, scale=1.0,
                    accum_out=sums[:, it, ci : ci + 1],
                )

            def emit_fin(it: int) -> None:
                sv = smalls.tile([P, 1], DT, tag="sv")
                nc.vector.reduce_sum(sv, sums[:, it, 0 : len(chunks_of(it))], axis=AX)
                rv = smalls.tile([P, 1], DT, tag="rv")
                nc.vector.reciprocal(rv, sv)
                vt = vtsp.tile([P, P], F16, tag="vts")
                nc.vector.tensor_scalar_mul(vt, vT_sb[:, it * P : (it + 1) * P], rv)
                vts_t[it] = vt

            def emit_y(g: int, jb: int) -> None:
                """y[:, jb] += vts.T @ exp over the 4 i-tiles of group g."""
                js = jb * 512
                yp = psY.tile([P, 512], DT, tag="y")
                for gi in range(G):
                    it = g * G + gi
                    nc.tensor.matmul(
                        yp, vts_t[it], exp_t[it][:, js : js + 512],
                        start=(gi == 0), stop=(gi == G - 1),
                    )
                if g == 0:
                    nc.vector.tensor_copy(y_sb[:, js : js + 512], yp)
                else:
                    nc.vector.tensor_add(
                        y_sb[:, js : js + 512], y_sb[:, js : js + 512], yp
                    )

            def emit_z(jb: int) -> None:
                # tail-only: ScalarE is idle after the last exp, so it does
                # the psum->sbuf flush; the +((x+zb)) add is split DVE/GpSimd
                js = jb * 512
                xzb = xch[jb]
                for h in range(2):
                    zp = psK.tile([P, 512], DT, tag="kt")
                    nc.tensor.matmul(zp, late["WzT"][:, h], y_sb[:, js : js + 512],
                                     start=True, stop=True)
                    zc = zst.tile([P, 512], DT, tag="zc")
                    nc.vector.tensor_add(zc, zp, xzb[h])
                    nc.sync.dma_start(out=z[h * P : (h + 1) * P, js : js + 512], in_=zc)

            # ================= schedule =================
            # warmup = group 0: q cols first (ktq moving operand), k/v for
            # group 0 just-in-time, interleaved with tile 0-3 ktq/exp; convs
            # for group 1 land inside group 0's stream.
            emit_x_cast(0)
            emit_q(0)
            emit_aux_cast(0)
            emit_k(0)
            emit_ktq_chunk(0, 0)
            emit_x_cast(1)
            emit_q(1)
            emit_x_cast(2)
            emit_q(2)
            emit_ktq_chunk(0, 1)
            emit_v(0, 0)
            emit_v(0, 1)
            emit_x_cast(3)
            emit_q(3)
            emit_x_cast(4)
            emit_q(4)
            emit_x_cast(5)
            emit_q(5)
            emit_ktq_chunk(0, 2)
            emit_x_cast(6)
            emit_q(6)
            emit_x_cast(7)
            emit_q(7)
            emit_ktq_chunk(0, 3)
            emit_fin(0)
            emit_ktq_chunk(1, 0)
            emit_aux_cast(1)
            emit_k(1)
            emit_ktq_chunk(1, 1)
            emit_v(1, 0)
            emit_ktq_chunk(1, 2)
            emit_fin(1)
            emit_v(1, 1)
            for it in (2, 3):
                for ci in range(3):
                    emit_ktq_chunk(it, ci)
                emit_fin(it)

            # steady state: group g runs its 12 ktq/exp chunks with fills:
            # y(g-1) x8, conv k/v for group g+1, aux prefetch for g+2.
            for g in range(1, NG):
                fills: list = []
                if 2 <= g <= 5:
                    for cb in (2 * g - 4, 2 * g - 3):
                        fills.append(lambda c=cb: emit_xzb(c))
                if g + 2 < NG:
                    fills.append(lambda a=g + 2: emit_aux_dma(a))
                fills.append(lambda gg=g: emit_y(gg - 1, 0))
                if g + 1 < NG:
                    fills.append(lambda a=g + 1: emit_aux_cast(a))
                    fills.append(lambda a=g + 1: emit_k(a))
                fills.append(lambda gg=g: emit_y(gg - 1, 1))
                if g + 1 < NG:
                    fills.append(lambda a=g + 1: emit_v(a, 0))
                fills.append(lambda gg=g: emit_y(gg - 1, 2))
                if g == 1:
                    fills.append(emit_late_consts)
                if g + 1 < NG:
                    fills.append(lambda a=g + 1: emit_v(a, 1))
                for jb in range(3, NJB):
                    fills.append(lambda gg=g, j=jb: emit_y(gg - 1, j))
                fi = 0
                nslots = G * 3
                nf = len(fills)
                acc = 0.0
                for t in range(G):
                    it = g * G + t
                    for ci in range(3):
                        emit_ktq_chunk(it, ci)
                        acc += nf / nslots
                        while fi < nf and fi < acc:
                            fills[fi]()
                            fi += 1
                    emit_fin(it)
                while fi < nf:
                    fills[fi]()
                    fi += 1

            # tail: y for the last group, z streamed per column block
            emit_y(NG - 1, 0)
            for jb in range(1, NJB):
                emit_y(NG - 1, jb)
                emit_z(jb - 1)
            emit_z(NJB - 1)
            if debug:
                def dump(dst, src_ap):
                    t = zst.tile([P, 512], DT, tag="zc")
                    nc.vector.tensor_copy(t, src_ap)
                    nc.sync.dma_start(out=dst, in_=t)
                for cb in range(NJB):
                    sl = slice(cb * 512, cb * 512 + 512)
                    dump(dbg["dq"][:, sl], q_sb[:, sl])
                    dump(dbg["dk"][:, sl], k_sb[:, sl])
                    dump(dbg["dvT"][:, sl], vT_sb[:, sl])
                    dump(dbg["dy"][:, sl], y_sb[:, sl])
                    dump(dbg["dexp0"][:, sl], exp_t[0][:, sl])
                sv2 = smalls.tile([P, NIT], DT, tag="dbg2")
                for it in range(NIT):
                    nc.vector.reduce_sum(sv2[:, it : it + 1], sums[:, it], axis=AX)
                nc.sync.dma_start(out=dbg["dsums"][:, :], in_=sv2)

    nc.compile()
    return nc


_NC = None


def _get_nc() -> bass.Bass:
    global _NC
    if _NC is None:
        _NC = build_module()
    return _NC


def _make_in_maps(inputs: dict[str, np.ndarray]) -> list[dict[str, np.ndarray]]:
    B = inputs["x"].shape[0]
    shared = {
        name: np.ascontiguousarray(np.asarray(inputs[name], dtype=np.float32))
        for name in ("Wq_b", "Wk_b", "Wv_b", "Wz_b")
    }
    for dev_name, host_name in (
        ("WqT_d", "Wq_w"), ("WkT_d", "Wk_w"), ("WvT_d", "Wv_w"), ("WzT_d", "Wz_w"),
    ):
        shared[dev_name] = np.ascontiguousarray(
            np.asarray(inputs[host_name], dtype=np.float32).T
        )
    in_maps = []
    for b in range(B):
        m = dict(shared)
        m["x"] = np.ascontiguousarray(
            np.asarray(inputs["x"][b], dtype=np.float32).reshape(C, HW)
        )
        m["aux"] = np.ascontiguousarray(
            np.asarray(inputs["aux"][b], dtype=np.float32).reshape(C, HW)
        )
        in_maps.append(m)
    return in_maps


def _install_ntff_hook_shim() -> None:
    """The agent image's antenv lacks axon_hooks; recreate it so
    run_bass_kernel_spmd(trace=True) can reach the libaxon NTFF profiler."""
    import types

    if "antenv.axon_hooks" in sys.modules:
        return
    import antenv

    mod = types.ModuleType("antenv.axon_hooks")
    state = {"hook": None}
    mod.set_axon_ntff_profile_hook = lambda h: state.__setitem__("hook", h)
    mod.get_axon_ntff_profile_hook = lambda: state["hook"]
    sys.modules["antenv.axon_hooks"] = mod
    antenv.axon_hooks = mod
    try:
        from trn_agent_boot.trn_boot import _ntff_profile_via_ctypes

        hook = _ntff_profile_via_ctypes("/opt/axon/libaxon_pjrt.so")
        if hook is not None:
            mod.set_axon_ntff_profile_hook(hook)
    except Exception as e:  # degrade to no tracing
        print(f"ntff hook unavailable: {e}", file=sys.stderr)


def run(inputs: dict[str, np.ndarray], trace: bool = False):
    """Run on the 8 NeuronCores; returns (output [8,256,64,64], BassKernelResults)."""
    from concourse.bass_utils import run_bass_kernel_spmd

    if trace:
        _install_ntff_hook_shim()
    nc = _get_nc()
    in_maps = _make_in_maps(inputs)
    res = run_bass_kernel_spmd(nc, in_maps, list(range(len(in_maps))), trace=trace)
    out = np.stack([r["z"].reshape(C, 64, 64) for r in res.results])
    return out.astype(np.float32), res


def kernel(**inputs: np.ndarray) -> np.ndarray:
    out, _ = run(inputs, trace=False)
    return out


if __name__ == "__main__":
    nc = build_module()
    print("module built ok")
